# revision 22
# baseline (speedup 1.0000x reference)
"""Causal multi-head attention on 8 Trainium2 NeuronCores.

Problem: B=4, S=2048, D=1024, H=16 heads, d_k=64, causal, fp32 in/out.

Sharding (host side): core c handles batch b=c//2 and head-half hh=c%2
(8 heads = 512 of the 1024 model dims). Each core computes its batch's
attention output for its 8 heads and the partial out-projection through
the matching 512 rows of Wo (+ bo/2, so the pair sums to +bo). The host
gathers by summing the two partials per batch. No collectives needed.

v3 schedule (fused pipeline, bf16 projections, fp8-DoubleRow AV + out
projection):
  K proj (all S) -> V proj -> Q proj chunk 0, then a software-pipelined
  task loop over (j, dc) = (512-query chunk, head-pair):
    pass1(j,dc): scores S^T[k,q] bf16 MMs (halves on disjoint PE row
        groups run concurrently) -> fp32 PSUM [128,1024] block-pairs ->
        causal tri-add -> one exp ACTIVATE per pair -> P^T in fp8e4m3
        (softmax renormalization cancels fp8 quantization bias)
    pass2(prev): AV via fp8 DoubleRow (two 128-key blocks per MM,
        2x PE rate; V pairs [128,2,8,80] with ones col -> rowsums),
        reciprocal_approx_fast on the rowsum row, gpsimd broadcast,
        normalized at -> fp8 [128,4,512] per chunk
    per chunk: out projection via fp8 DoubleRow (two head-pairs per
        MM) + bo/2 -> DMA out
  Q proj of chunk j+1 between pass1(j) and pass2(j) keeps the PE busy
  while ACT runs exp(j); out-proj of chunk j overlaps exp(j+1).
"""
import sys

for _p in ("/opt/trn_rl_repo",):
    if _p not in sys.path:
        sys.path.insert(0, _p)

import numpy as np
import ml_dtypes

import concourse.bass as bass
import concourse.tile as tile
from concourse import bacc, bass_utils, library_config, mybir

F32 = mybir.dt.float32
BF16 = mybir.dt.bfloat16
FP8 = mybir.dt.float8e4
EXPF = mybir.ActivationFunctionType.Exp
ADD = mybir.AluOpType.add
MULT = mybir.AluOpType.mult
DR = mybir.MatmulPerfMode.DoubleRow

D = 1024          # model dim
S = 2048          # sequence length
DL = 512          # local head dims (8 heads x 64)
NH = 8            # local heads
NC_ = 8           # cores
NEG = -1.0e30
BF = ml_dtypes.bfloat16
F8 = ml_dtypes.float8_e4m3

_CACHE = {}
TRACE = False
last_results = None


def build_program():
    nc = bacc.Bacc("TRN2", target_bir_lowering=False, debug=False)

    xt_d = nc.dram_tensor("xt", [D, S], BF16, kind="ExternalInput").ap()
    wq_d = nc.dram_tensor("wq", [D, DL], BF16, kind="ExternalInput").ap()
    wk_d = nc.dram_tensor("wk", [D, DL], BF16, kind="ExternalInput").ap()
    wv_d = nc.dram_tensor("wv", [D, DL], BF16, kind="ExternalInput").ap()
    wo_d = nc.dram_tensor("wo", [DL, D], BF16, kind="ExternalInput").ap()
    bq_d = nc.dram_tensor("bq2", [128, 4], F32, kind="ExternalInput").ap()
    bk_d = nc.dram_tensor("bk2", [128, 4], F32, kind="ExternalInput").ap()
    bv_d = nc.dram_tensor("bv", [DL], F32, kind="ExternalInput").ap()
    bo_d = nc.dram_tensor("boh", [D], F32, kind="ExternalInput").ap()
    tri_d = nc.dram_tensor("tri", [128, 128], F32, kind="ExternalInput").ap()
    out_d = nc.dram_tensor("out", [S, D], F32, kind="ExternalOutput").ap()

    xt_r = xt_d.rearrange("(c p) n -> c p n", p=128)    # 8 din-chunks
    wq_r = wq_d.rearrange("(c p) n -> c p n", p=128)
    wk_r = wk_d.rearrange("(c p) n -> c p n", p=128)
    wv_r = wv_d.rearrange("(c p) n -> c p n", p=128)
    wo_r = wo_d.rearrange("(c p) n -> c p n", p=128)    # 4 head-pair chunks

    with tile.TileContext(nc) as tc:
        nc.gpsimd.load_library(library_config.attn)

        consts = tc.alloc_tile_pool(name="consts", bufs=1)
        xtp = tc.alloc_tile_pool(name="xtp", bufs=1)
        qkp = tc.alloc_tile_pool(name="qkp", bufs=1)
        vp = tc.alloc_tile_pool(name="vp", bufs=1)
        wqp = tc.alloc_tile_pool(name="wqp", bufs=1)
        wop = tc.alloc_tile_pool(name="wop", bufs=1)
        # wk/wv on top of the pool stack: released after K/V projections
        # so the loop pools below reuse their space
        wkp = tc.alloc_tile_pool(name="wkp", bufs=1)
        wvp = tc.alloc_tile_pool(name="wvp", bufs=1)

        # ---- weight DMAs on the gpsimd queue (wk + K biases first) ----
        wkt = [wkp.tile([128, DL], BF16, tag=f"wk{i}", name=f"wk{i}")
               for i in range(8)]
        for c in range(8):
            nc.gpsimd.dma_start(wkt[c], wk_r[c])
        bk2 = consts.tile([128, 4], F32, tag="bk2", name="bk2")
        nc.gpsimd.dma_start(bk2, bk_d)
        bq2 = consts.tile([128, 4], F32, tag="bq2", name="bq2")
        nc.gpsimd.dma_start(bq2, bq_d)
        wvt = [wvp.tile([128, DL], BF16, tag=f"wv{i}", name=f"wv{i}")
               for i in range(8)]
        for c in range(8):
            nc.gpsimd.dma_start(wvt[c], wv_r[c])
        bvb = consts.tile([128, DL], BF16, tag="bvb", name="bvb")
        nc.gpsimd.dma_start(
            bvb,
            bass.AP(tensor=bv_d.tensor, offset=bv_d.offset,
                    ap=[[0, 128]] + bv_d.ap))

        # ---- x^T in S-quarters: scalar ring gets the early columns ----
        xt = [xtp.tile([128, S], BF16, tag=f"xt{i}", name=f"xt{i}")
              for i in range(8)]
        for q in range(2):
            for c in range(8):
                nc.scalar.dma_start(xt[c][:, q * 512:(q + 1) * 512],
                                    xt_r[c][:, q * 512:(q + 1) * 512])
        for q in range(2, 4):
            for c in range(8):
                nc.sync.dma_start(xt[c][:, q * 512:(q + 1) * 512],
                                  xt_r[c][:, q * 512:(q + 1) * 512])

        # ---- remaining weights/consts (gpsimd queue) ----
        wqt = [wqp.tile([128, DL], BF16, tag=f"wq{i}", name=f"wq{i}")
               for i in range(8)]
        for c in range(8):
            nc.gpsimd.dma_start(wqt[c], wq_r[c])
        wo2 = wop.tile([128, 4, D], BF16, tag="wo2", name="wo2")
        for hc in range(4):
            nc.gpsimd.dma_start(wo2[:, hc, :], wo_r[hc])
        bob = consts.tile([128, D], F32, tag="bob", name="bob")
        nc.gpsimd.dma_start(
            bob,
            bass.AP(tensor=bo_d.tensor, offset=bo_d.offset,
                    ap=[[0, 128]] + bo_d.ap))
        tri = consts.tile([128, 128], F32, tag="tri", name="tri")
        nc.gpsimd.dma_start(tri, tri_d)
        ones8 = consts.tile([128, NH], BF16, tag="ones8", name="ones8")
        nc.vector.memset(ones8[:], 1.0)

        kt = [qkp.tile([128, S], BF16, tag=f"kt{i}", name=f"kt{i}")
              for i in range(4)]
        qt = [qkp.tile([128, S], BF16, tag=f"qt{i}", name=f"qt{i}")
              for i in range(4)]
        # V per key block: cols 0:64 = head dims, col 64 = ones (rowsum)
        v = [vp.tile([128, NH, 65], BF16, tag=f"v{i}", name=f"v{i}")
             for i in range(16)]

        psp = tc.alloc_tile_pool(name="psp", bufs=2, space="PSUM")
        s4p = tc.alloc_tile_pool(name="s4p", bufs=3, space="PSUM")

        # ================= K projection (all S) ======================
        for jq in range(4):
            for dc in range(4):
                ps = psp.tile([128, 512], F32, tag="ps", name="psK")
                for c in range(8):
                    nc.tensor.matmul(
                        ps[:],
                        wkt[c][:, dc * 128:(dc + 1) * 128],
                        xt[c][:, jq * 512:(jq + 1) * 512],
                        start=(c == 0), stop=(c == 7))
                nc.vector.tensor_scalar_add(
                    kt[dc][:, jq * 512:(jq + 1) * 512],
                    ps[:], bk2[:, dc:dc + 1])

        # ================= V projection ==============================
        bvb3 = bvb[:].rearrange("p (h d) -> p h d", h=NH)
        for sb in range(16):
            ps = psp.tile([128, 512], F32, tag="ps", name="psV")
            for c in range(8):
                nc.tensor.matmul(
                    ps[:],
                    xt[c][:, sb * 128:(sb + 1) * 128],
                    wvt[c][:],
                    start=(c == 0), stop=(c == 7))
            nc.vector.tensor_tensor(
                v[sb][:, :, 0:64],
                ps[:].rearrange("p (h d) -> p h d", h=NH),
                bvb3, op=ADD)
            nc.vector.tensor_copy(v[sb][:, :, 64], ones8[:])
        wvp.release()
        wkp.release()

        # loop-phase pools allocated after wk/wv release (space reuse)
        atp = tc.alloc_tile_pool(name="atp", bufs=2)
        ptp = tc.alloc_tile_pool(name="ptp", bufs=2)
        rcp = tc.alloc_tile_pool(name="rcp", bufs=2)
        bcp = tc.alloc_tile_pool(name="bcp", bufs=2)
        outp = tc.alloc_tile_pool(name="outp", bufs=2)

        def qproj(j):
            for dc in range(4):
                ps = psp.tile([128, 512], F32, tag="ps", name="psQ")
                for c in range(8):
                    nc.tensor.matmul(
                        ps[:],
                        wqt[c][:, dc * 128:(dc + 1) * 128],
                        xt[c][:, j * 512:(j + 1) * 512],
                        start=(c == 0), stop=(c == 7))
                nc.vector.tensor_scalar_add(
                    qt[dc][:, j * 512:(j + 1) * 512],
                    ps[:], bq2[:, dc:dc + 1])

        def pass1_dc(j, dc):
            """Scores + exp for all key-block pairs of (q-chunk j, head
            pair dc); returns pt tiles (fp8) keyed (g2, half)."""
            pts = {}
            for g2 in range(2 * j + 2):
                s4s = {}
                for half in range(2):
                    s4s[half] = s4p.tile([128, 1024], F32,
                                         tag="s4", name="s4")
                lss = []
                for kk in range(2):
                    kb = 2 * g2 + kk
                    ls = max(0, 128 * kb - 512 * j)
                    lss.append(ls)
                    for half in range(2):
                        pr = 64 * half
                        nc.tensor.matmul(
                            s4s[half][:, 512 * kk + ls:512 * (kk + 1)],
                            kt[dc][pr:pr + 64, 128 * kb:128 * (kb + 1)],
                            qt[dc][pr:pr + 64, 512 * j + ls:512 * (j + 1)],
                            start=True, stop=True)
                diag = (2 * g2 >= 4 * j)     # both blocks on the diagonal
                for half in range(2):
                    s4 = s4s[half]
                    if diag:
                        for kk in range(2):
                            p0 = 512 * kk + lss[kk]
                            sl = s4[:, p0:p0 + 128]
                            nc.vector.tensor_tensor(sl, sl, tri[:], op=ADD)
                    pt = ptp.tile([128, 1024], BF16,
                                  tag=f"pt{g2}_{half}", name="pt")
                    pts[(g2, half)] = pt
                    nc.scalar.activation(
                        pt[:, lss[0]:1024], s4[:, lss[0]:1024],
                        EXPF, scale=0.125)
            return pts

        def pass2_dc(j, dc, pts, att):
            """AV chains (bf16) + normalized at."""
            for half in range(2):
                pr = 64 * half
                av = psp.tile([128, 512], F32, tag="ps", name="av")
                first = True
                for g2 in range(2 * j + 2):
                    pt = pts[(g2, half)]
                    for kk in range(2):
                        kb = 2 * g2 + kk
                        ls = max(0, 128 * kb - 512 * j)
                        nc.tensor.matmul(
                            av[0:65, ls:512],
                            v[kb][:, 2 * dc + half, :],
                            pt[:, 512 * kk + ls:512 * (kk + 1)],
                            start=first, stop=True,
                            skip_group_check=not first)
                        first = False
                rsum = rcp.tile([1, 512], F32, tag="rsum", name="rsum")
                nc.vector.tensor_copy(rsum[:], av[64:65, :])
                rec = rcp.tile([1, 512], F32, tag="rec", name="rec")
                nc.vector.reciprocal_approx_fast(rec[:], rsum[:])
                bc = bcp.tile([64, 512], F32, tag=f"bc{half}", name="bc")
                nc.gpsimd.partition_broadcast(bc[:], rec[:])
                nc.vector.tensor_tensor(
                    att[pr:pr + 64, dc, :], av[0:64, :], bc[:], op=MULT)

        def phase_d(j, att):
            for si in range(4):
                sb = 4 * j + si
                ot = outp.tile([128, D], F32, tag="ot", name="ot")
                for n in range(2):
                    ps = psp.tile([128, 512], F32, tag="ps", name="psD")
                    for hc in range(4):
                        nc.tensor.matmul(
                            ps[:],
                            att[:, hc, si * 128:(si + 1) * 128],
                            wo2[:, hc, n * 512:(n + 1) * 512],
                            start=(hc == 0), stop=(hc == 3))
                    nc.vector.tensor_tensor(
                        ot[:, n * 512:(n + 1) * 512], ps[:],
                        bob[:, n * 512:(n + 1) * 512], op=ADD)
                nc.sync.dma_start(out_d[sb * 128:(sb + 1) * 128, :], ot[:])

        # ================= fused attention pipeline ==================
        qproj(0)
        tasks = [(j, dc) for j in range(4) for dc in range(4)]
        prev = None
        cur_at = None
        for (j, dc) in tasks:
            if dc == 0:
                cur_at = atp.tile([128, 4, 512], BF16, tag="att",
                                  name="att")
            pts = pass1_dc(j, dc)
            if j < 3 and dc == 3:
                qproj(j + 1)
            if prev is not None:
                pj, pdc, ppts, pat = prev
                pass2_dc(pj, pdc, ppts, pat)
                if pdc == 3:
                    phase_d(pj, pat)
            prev = (j, dc, pts, cur_at)
        pj, pdc, ppts, pat = prev
        pass2_dc(pj, pdc, ppts, pat)
        phase_d(pj, pat)

        outp.release()
        bcp.release()
        rcp.release()
        ptp.release()
        atp.release()
        s4p.release()
        psp.release()
        wop.release()
        wqp.release()
        vp.release()
        qkp.release()
        xtp.release()
        consts.release()

    nc.compile()
    return nc


def make_in_maps(x, Wq, bq, Wk, bk, Wv, bv, Wo, bo):
    x = np.asarray(x, np.float32)
    Wq, bq = np.asarray(Wq, np.float32), np.asarray(bq, np.float32)
    Wk, bk = np.asarray(Wk, np.float32), np.asarray(bk, np.float32)
    Wv, bv = np.asarray(Wv, np.float32), np.asarray(bv, np.float32)
    Wo, bo = np.asarray(Wo, np.float32), np.asarray(bo, np.float32)

    k = np.arange(128)[:, None]
    c = np.arange(128)[None, :]
    tri = np.where(c >= k, 0.0, NEG).astype(np.float32)
    boh = (bo * 0.5).astype(np.float32)

    in_maps = []
    for core in range(NC_):
        b, hh = core // 2, core % 2
        sl = slice(hh * DL, (hh + 1) * DL)
        in_maps.append({
            "xt": np.ascontiguousarray(x[b].T).astype(BF),
            "wq": np.ascontiguousarray(Wq[:, sl]).astype(BF),
            "wk": np.ascontiguousarray(Wk[:, sl]).astype(BF),
            "wv": np.ascontiguousarray(Wv[:, sl]).astype(BF),
            "wo": np.ascontiguousarray(Wo[sl, :]).astype(BF),
            "bq2": np.ascontiguousarray(bq[sl].reshape(4, 128).T),
            "bk2": np.ascontiguousarray(bk[sl].reshape(4, 128).T),
            "bv": np.ascontiguousarray(bv[sl]),
            "boh": boh,
            "tri": tri,
        })
    return in_maps


def kernel(x, Wq, bq, Wk, bk, Wv, bv, Wo, bo):
    global last_results
    if "nc" not in _CACHE:
        _CACHE["nc"] = build_program()
    nc = _CACHE["nc"]
    in_maps = make_in_maps(x, Wq, bq, Wk, bk, Wv, bv, Wo, bo)
    res = bass_utils.run_bass_kernel_spmd(
        nc, in_maps, core_ids=list(range(NC_)), trace=TRACE)
    last_results = res
    B = 4
    out = np.empty((B, S, D), np.float32)
    for b in range(B):
        out[b] = res.results[2 * b]["out"] + res.results[2 * b + 1]["out"]
    return out


# revision 24
# speedup vs baseline: 1.0205x; 1.0205x over previous
"""Causal multi-head attention on 8 Trainium2 NeuronCores.

Problem: B=4, S=2048, D=1024, H=16 heads, d_k=64, causal, fp32 in/out.

Sharding (host side): core c handles batch b=c//2 and head-half hh=c%2
(8 heads = 512 of the 1024 model dims). Each core computes its batch's
attention output for its 8 heads and the partial out-projection through
the matching 512 rows of Wo (+ bo/2, so the pair sums to +bo). The host
gathers by summing the two partials per batch. No collectives needed.

v3 schedule (fused pipeline, bf16 projections, fp8-DoubleRow AV + out
projection):
  K proj (all S) -> V proj -> Q proj chunk 0, then a software-pipelined
  task loop over (j, dc) = (512-query chunk, head-pair):
    pass1(j,dc): scores S^T[k,q] bf16 MMs (halves on disjoint PE row
        groups run concurrently) -> fp32 PSUM [128,1024] block-pairs ->
        causal tri-add -> one exp ACTIVATE per pair -> P^T in fp8e4m3
        (softmax renormalization cancels fp8 quantization bias)
    pass2(prev): AV via fp8 DoubleRow (two 128-key blocks per MM,
        2x PE rate; V pairs [128,2,8,80] with ones col -> rowsums),
        reciprocal_approx_fast on the rowsum row, gpsimd broadcast,
        normalized at -> fp8 [128,4,512] per chunk
    per chunk: out projection via fp8 DoubleRow (two head-pairs per
        MM) + bo/2 -> DMA out
  Q proj of chunk j+1 between pass1(j) and pass2(j) keeps the PE busy
  while ACT runs exp(j); out-proj of chunk j overlaps exp(j+1).
"""
import sys

for _p in ("/opt/trn_rl_repo",):
    if _p not in sys.path:
        sys.path.insert(0, _p)

import numpy as np
import ml_dtypes

import concourse.bass as bass
import concourse.tile as tile
from concourse import bacc, bass_utils, library_config, mybir

F32 = mybir.dt.float32
BF16 = mybir.dt.bfloat16
FP8 = mybir.dt.float8e4
EXPF = mybir.ActivationFunctionType.Exp
ADD = mybir.AluOpType.add
MULT = mybir.AluOpType.mult
DR = mybir.MatmulPerfMode.DoubleRow

D = 1024          # model dim
S = 2048          # sequence length
DL = 512          # local head dims (8 heads x 64)
NH = 8            # local heads
NC_ = 8           # cores
NEG = -1.0e30
BF = ml_dtypes.bfloat16
F8 = ml_dtypes.float8_e4m3

_CACHE = {}
TRACE = False
last_results = None


def build_program():
    nc = bacc.Bacc("TRN2", target_bir_lowering=False, debug=False)

    xt_d = nc.dram_tensor("xt", [D, S], BF16, kind="ExternalInput").ap()
    wq_d = nc.dram_tensor("wq", [D, DL], BF16, kind="ExternalInput").ap()
    wk_d = nc.dram_tensor("wk", [D, DL], BF16, kind="ExternalInput").ap()
    wv_d = nc.dram_tensor("wv", [D, DL], BF16, kind="ExternalInput").ap()
    wo_d = nc.dram_tensor("wo", [DL, D], BF16, kind="ExternalInput").ap()
    bq_d = nc.dram_tensor("bq2", [128, 4], F32, kind="ExternalInput").ap()
    bk_d = nc.dram_tensor("bk2", [128, 4], F32, kind="ExternalInput").ap()
    bv_d = nc.dram_tensor("bv", [DL], F32, kind="ExternalInput").ap()
    bo_d = nc.dram_tensor("boh", [D], F32, kind="ExternalInput").ap()
    tri_d = nc.dram_tensor("tri", [128, 128], F32, kind="ExternalInput").ap()
    out_d = nc.dram_tensor("out", [S, D], F32, kind="ExternalOutput").ap()

    xt_r = xt_d.rearrange("(c p) n -> c p n", p=128)    # 8 din-chunks
    wq_r = wq_d.rearrange("(c p) n -> c p n", p=128)
    wk_r = wk_d.rearrange("(c p) n -> c p n", p=128)
    wv_r = wv_d.rearrange("(c p) n -> c p n", p=128)
    wo_r = wo_d.rearrange("(c p) n -> c p n", p=128)    # 4 head-pair chunks

    with tile.TileContext(nc) as tc:
        nc.gpsimd.load_library(library_config.attn)

        consts = tc.alloc_tile_pool(name="consts", bufs=1)
        xtp = tc.alloc_tile_pool(name="xtp", bufs=1)
        qkp = tc.alloc_tile_pool(name="qkp", bufs=1)
        vp = tc.alloc_tile_pool(name="vp", bufs=1)
        wqp = tc.alloc_tile_pool(name="wqp", bufs=1)
        wop = tc.alloc_tile_pool(name="wop", bufs=1)
        atp = tc.alloc_tile_pool(name="atp", bufs=2)
        ptp = tc.alloc_tile_pool(name="ptp", bufs=2)
        rcp = tc.alloc_tile_pool(name="rcp", bufs=1)
        bcp = tc.alloc_tile_pool(name="bcp", bufs=1)
        outp = tc.alloc_tile_pool(name="outp", bufs=2)
        # wk/wv on top of the pool stack: released once the last backfilled
        # projection chains have been emitted (during chunk j=2)
        wkp = tc.alloc_tile_pool(name="wkp", bufs=1)
        wvp = tc.alloc_tile_pool(name="wvp", bufs=1)

        # ---- weight DMAs on the gpsimd queue (wk + K biases first) ----
        wkt = [wkp.tile([128, DL], BF16, tag=f"wk{i}", name=f"wk{i}")
               for i in range(8)]
        for c in range(8):
            nc.gpsimd.dma_start(wkt[c], wk_r[c])
        bk2 = consts.tile([128, 4], F32, tag="bk2", name="bk2")
        nc.gpsimd.dma_start(bk2, bk_d)
        bq2 = consts.tile([128, 4], F32, tag="bq2", name="bq2")
        nc.gpsimd.dma_start(bq2, bq_d)
        wqt = [wqp.tile([128, DL], BF16, tag=f"wq{i}", name=f"wq{i}")
               for i in range(8)]
        for c in range(8):
            nc.gpsimd.dma_start(wqt[c], wq_r[c])
        wvt = [wvp.tile([128, DL], BF16, tag=f"wv{i}", name=f"wv{i}")
               for i in range(8)]
        for c in range(8):
            nc.gpsimd.dma_start(wvt[c], wv_r[c])
        bvb = consts.tile([128, DL], BF16, tag="bvb", name="bvb")
        nc.gpsimd.dma_start(
            bvb,
            bass.AP(tensor=bv_d.tensor, offset=bv_d.offset,
                    ap=[[0, 128]] + bv_d.ap))
        tri = consts.tile([128, 128], F32, tag="tri", name="tri")
        nc.gpsimd.dma_start(tri, tri_d)

        # ---- x^T in S-quarters: scalar ring gets the early columns ----
        xt = [xtp.tile([128, S], BF16, tag=f"xt{i}", name=f"xt{i}")
              for i in range(8)]
        for q in range(2):
            for c in range(8):
                nc.scalar.dma_start(xt[c][:, q * 512:(q + 1) * 512],
                                    xt_r[c][:, q * 512:(q + 1) * 512])
        for q in range(2, 4):
            for c in range(8):
                nc.sync.dma_start(xt[c][:, q * 512:(q + 1) * 512],
                                  xt_r[c][:, q * 512:(q + 1) * 512])

        # ---- remaining weights/consts (gpsimd queue) ----
        wo2 = wop.tile([128, 4, D], BF16, tag="wo2", name="wo2")
        for hc in range(4):
            nc.gpsimd.dma_start(wo2[:, hc, :], wo_r[hc])
        bob = consts.tile([128, D], F32, tag="bob", name="bob")
        nc.gpsimd.dma_start(
            bob,
            bass.AP(tensor=bo_d.tensor, offset=bo_d.offset,
                    ap=[[0, 128]] + bo_d.ap))
        ones8 = consts.tile([128, NH], BF16, tag="ones8", name="ones8")
        nc.vector.memset(ones8[:], 1.0)

        kt = [qkp.tile([128, S], BF16, tag=f"kt{i}", name=f"kt{i}")
              for i in range(4)]
        qt = [qkp.tile([128, S], BF16, tag=f"qt{i}", name=f"qt{i}")
              for i in range(4)]
        # V per key block: cols 0:64 = head dims, col 64 = ones (rowsum)
        v = [vp.tile([128, NH, 65], BF16, tag=f"v{i}", name=f"v{i}")
             for i in range(16)]

        psp = tc.alloc_tile_pool(name="psp", bufs=2, space="PSUM")
        s4p = tc.alloc_tile_pool(name="s4p", bufs=3, space="PSUM")

        bvb3 = bvb[:].rearrange("p (h d) -> p h d", h=NH)

        def k_chain(jq, dc):
            ps = psp.tile([128, 512], F32, tag="ps", name="psK")
            for c in range(8):
                nc.tensor.matmul(
                    ps[:],
                    wkt[c][:, dc * 128:(dc + 1) * 128],
                    xt[c][:, jq * 512:(jq + 1) * 512],
                    start=(c == 0), stop=(c == 7))
            nc.vector.tensor_scalar_add(
                kt[dc][:, jq * 512:(jq + 1) * 512],
                ps[:], bk2[:, dc:dc + 1])

        def v_chain(sb):
            ps = psp.tile([128, 512], F32, tag="ps", name="psV")
            for c in range(8):
                nc.tensor.matmul(
                    ps[:],
                    xt[c][:, sb * 128:(sb + 1) * 128],
                    wvt[c][:],
                    start=(c == 0), stop=(c == 7))
            nc.vector.tensor_tensor(
                v[sb][:, :, 0:64],
                ps[:].rearrange("p (h d) -> p h d", h=NH),
                bvb3, op=ADD)
            nc.vector.tensor_copy(v[sb][:, :, 64], ones8[:])

        # warmup: only what chunk j=0 needs (keys/values 0-511)
        for dc in range(4):
            k_chain(0, dc)
        for sb in range(4):
            v_chain(sb)
        # remaining projection chains, backfilled into chunk j's tasks
        # (2 per task) so they overlap the exp stream
        backfill = {
            jn: [lambda jq=jn, dc=dc: k_chain(jq, dc) for dc in range(4)]
            + [lambda sb=sb: v_chain(sb) for sb in range(4 * jn, 4 * jn + 4)]
            for jn in range(1, 4)
        }

        def qproj(j):
            for dc in range(4):
                ps = psp.tile([128, 512], F32, tag="ps", name="psQ")
                for c in range(8):
                    nc.tensor.matmul(
                        ps[:],
                        wqt[c][:, dc * 128:(dc + 1) * 128],
                        xt[c][:, j * 512:(j + 1) * 512],
                        start=(c == 0), stop=(c == 7))
                nc.vector.tensor_scalar_add(
                    qt[dc][:, j * 512:(j + 1) * 512],
                    ps[:], bq2[:, dc:dc + 1])

        def pass1_dc(j, dc):
            """Scores + exp for all key-block pairs of (q-chunk j, head
            pair dc); returns pt tiles (fp8) keyed (g2, half)."""
            pts = {}
            for g2 in range(2 * j + 2):
                s4s = {}
                for half in range(2):
                    s4s[half] = s4p.tile([128, 1024], F32,
                                         tag="s4", name="s4")
                lss = []
                for kk in range(2):
                    kb = 2 * g2 + kk
                    ls = max(0, 128 * kb - 512 * j)
                    lss.append(ls)
                    for half in range(2):
                        pr = 64 * half
                        nc.tensor.matmul(
                            s4s[half][:, 512 * kk + ls:512 * (kk + 1)],
                            kt[dc][pr:pr + 64, 128 * kb:128 * (kb + 1)],
                            qt[dc][pr:pr + 64, 512 * j + ls:512 * (j + 1)],
                            start=True, stop=True)
                diag = (2 * g2 >= 4 * j)     # both blocks on the diagonal
                for half in range(2):
                    s4 = s4s[half]
                    if diag:
                        for kk in range(2):
                            p0 = 512 * kk + lss[kk]
                            sl = s4[:, p0:p0 + 128]
                            nc.vector.tensor_tensor(sl, sl, tri[:], op=ADD)
                    pt = ptp.tile([128, 1024], BF16,
                                  tag=f"pt{g2}_{half}", name="pt")
                    pts[(g2, half)] = pt
                    nc.scalar.activation(
                        pt[:, lss[0]:1024], s4[:, lss[0]:1024],
                        EXPF, scale=0.125)
            return pts

        def pass2_dc(j, dc, pts, att):
            """AV chains (bf16) + normalized at."""
            for half in range(2):
                pr = 64 * half
                av = psp.tile([128, 512], F32, tag="ps", name="av")
                first = True
                for g2 in range(2 * j + 2):
                    pt = pts[(g2, half)]
                    for kk in range(2):
                        kb = 2 * g2 + kk
                        ls = max(0, 128 * kb - 512 * j)
                        nc.tensor.matmul(
                            av[0:65, ls:512],
                            v[kb][:, 2 * dc + half, :],
                            pt[:, 512 * kk + ls:512 * (kk + 1)],
                            start=first, stop=True,
                            skip_group_check=not first)
                        first = False
                rsum = rcp.tile([1, 512], F32, tag="rsum", name="rsum")
                nc.vector.tensor_copy(rsum[:], av[64:65, :])
                rec = rcp.tile([1, 512], F32, tag="rec", name="rec")
                nc.vector.reciprocal_approx_fast(rec[:], rsum[:])
                bc = bcp.tile([64, 512], F32, tag=f"bc{half}", name="bc")
                nc.gpsimd.partition_broadcast(bc[:], rec[:])
                nc.vector.tensor_tensor(
                    att[pr:pr + 64, dc, :], av[0:64, :], bc[:], op=MULT)

        def phase_d(j, att):
            for si in range(4):
                sb = 4 * j + si
                ot = outp.tile([128, D], F32, tag="ot", name="ot")
                for n in range(2):
                    ps = psp.tile([128, 512], F32, tag="ps", name="psD")
                    for hc in range(4):
                        nc.tensor.matmul(
                            ps[:],
                            att[:, hc, si * 128:(si + 1) * 128],
                            wo2[:, hc, n * 512:(n + 1) * 512],
                            start=(hc == 0), stop=(hc == 3))
                    nc.vector.tensor_tensor(
                        ot[:, n * 512:(n + 1) * 512], ps[:],
                        bob[:, n * 512:(n + 1) * 512], op=ADD)
                nc.sync.dma_start(out_d[sb * 128:(sb + 1) * 128, :], ot[:])

        # ================= fused attention pipeline ==================
        qproj(0)
        tasks = [(j, dc) for j in range(4) for dc in range(4)]
        prev = None
        cur_at = None
        for (j, dc) in tasks:
            if dc == 0:
                cur_at = atp.tile([128, 4, 512], BF16, tag="att",
                                  name="att")
            pts = pass1_dc(j, dc)
            if j < 3:
                for th in backfill[j + 1][2 * dc:2 * dc + 2]:
                    th()
            if j < 3 and dc == 3:
                qproj(j + 1)
                if j == 2:
                    wvp.release()
                    wkp.release()
            if prev is not None:
                pj, pdc, ppts, pat = prev
                pass2_dc(pj, pdc, ppts, pat)
                if pdc == 3:
                    phase_d(pj, pat)
            prev = (j, dc, pts, cur_at)
        pj, pdc, ppts, pat = prev
        pass2_dc(pj, pdc, ppts, pat)
        phase_d(pj, pat)

        s4p.release()
        psp.release()
        outp.release()
        bcp.release()
        rcp.release()
        ptp.release()
        atp.release()
        wop.release()
        wqp.release()
        vp.release()
        qkp.release()
        xtp.release()
        consts.release()

    nc.compile()
    return nc


def make_in_maps(x, Wq, bq, Wk, bk, Wv, bv, Wo, bo):
    x = np.asarray(x, np.float32)
    Wq, bq = np.asarray(Wq, np.float32), np.asarray(bq, np.float32)
    Wk, bk = np.asarray(Wk, np.float32), np.asarray(bk, np.float32)
    Wv, bv = np.asarray(Wv, np.float32), np.asarray(bv, np.float32)
    Wo, bo = np.asarray(Wo, np.float32), np.asarray(bo, np.float32)

    k = np.arange(128)[:, None]
    c = np.arange(128)[None, :]
    tri = np.where(c >= k, 0.0, NEG).astype(np.float32)
    boh = (bo * 0.5).astype(np.float32)

    in_maps = []
    for core in range(NC_):
        b, hh = core // 2, core % 2
        sl = slice(hh * DL, (hh + 1) * DL)
        in_maps.append({
            "xt": np.ascontiguousarray(x[b].T).astype(BF),
            "wq": np.ascontiguousarray(Wq[:, sl]).astype(BF),
            "wk": np.ascontiguousarray(Wk[:, sl]).astype(BF),
            "wv": np.ascontiguousarray(Wv[:, sl]).astype(BF),
            "wo": np.ascontiguousarray(Wo[sl, :]).astype(BF),
            "bq2": np.ascontiguousarray(bq[sl].reshape(4, 128).T),
            "bk2": np.ascontiguousarray(bk[sl].reshape(4, 128).T),
            "bv": np.ascontiguousarray(bv[sl]),
            "boh": boh,
            "tri": tri,
        })
    return in_maps


def kernel(x, Wq, bq, Wk, bk, Wv, bv, Wo, bo):
    global last_results
    if "nc" not in _CACHE:
        _CACHE["nc"] = build_program()
    nc = _CACHE["nc"]
    in_maps = make_in_maps(x, Wq, bq, Wk, bk, Wv, bv, Wo, bo)
    res = bass_utils.run_bass_kernel_spmd(
        nc, in_maps, core_ids=list(range(NC_)), trace=TRACE)
    last_results = res
    B = 4
    out = np.empty((B, S, D), np.float32)
    for b in range(B):
        out[b] = res.results[2 * b]["out"] + res.results[2 * b + 1]["out"]
    return out
